# revision 1
# baseline (speedup 1.0000x reference)
"""Trainium2 Bass kernel for a dynamic-range compressor (nn_Compressor).

Reference semantics (fp32):
    audio_db = 20*log10(|audio| + 1e-5)
    gr_db    = max((threshold - audio_db) * (1 - 1/ratio), 0)
    scan:    g[t] = g[t-1] + (1-c)*(gr_db[t] - g[t-1]),  c = attack if gr_db[t] > g[t-1] else release
    out      = audio * 10^(-g/20)

Parallelization: the smoother is strongly contracting (error decays by a
factor of max(attack, release) per step), so a chunked scan with a short
warmup prefix converges to fp32-exact. Each of the 8 cores handles a
contiguous 512K-sample shard; inside a core the shard is split into 128
partition-rows of 4096 samples, each with a W-sample warmup halo.

Per-row recurrence, branchless form:
    g[t] = max(attack*g[t-1] + (1-attack)*x[t], release*g[t-1] + (1-release)*x[t])
Solved with the hardware scan instruction (tensor_tensor_scan):
  1. predictor scan: m[t] = max(release*m[t-1], x[t])   (op0=mult, op1=max)
  2. branch mask from sign(x[t] - m[t-1]) -> coeff / (1-coeff) tiles
  3. linear scan:   g[t] = coeff[t]*g[t-1] + (1-coeff[t])*x[t]  (mult, add)
  4. optional second refinement iteration (branch from g instead of m)
One refinement iteration gives ~2e-6 relative output error vs the
sequential reference (validated in numpy); two give ~2e-8.
"""

import math
import os

import numpy as np

import sys

if "/opt/trn_rl_repo" not in sys.path:
    sys.path.insert(0, "/opt/trn_rl_repo")

P = 128            # SBUF partitions
C = 4096           # valid samples per partition-row
W = 32             # warmup samples per row
NCORES = 8
SHARD = P * C      # samples per core
T_TOTAL = NCORES * SHARD
N_ITER = int(os.environ.get("COMP_N_ITER", "1"))
NBLK = int(os.environ.get("COMP_NBLK", "6"))
PAD_VAL = 1e9      # |audio| huge -> gain_reduction = 0 -> matches g=0 initial state


def _register_custom_ops():
    """Fused DVE ops for the branch-select chains, registered at runtime:
      COMP_COEFF_SEL: out = select(in0 > in1, s0, s1)
      COMP_D1_SEL:    out = select(in0 > in1, s0, s1) * in0
    """
    import concourse.dve_ops as dve_ops
    from concourse.dve_spec import Spec, Src0, Src1, C0, C1, select, lower
    from concourse.dve_uop import DveOpSpec

    existing = {o.name: o for o in dve_ops.OPS}
    if "COMP_COEFF_SEL" in existing:
        return existing["COMP_COEFF_SEL"], existing["COMP_D1_SEL"]

    def mk(name, body, reference):
        spec = Spec(body=body, reference=reference)
        row = dve_ops._CUSTOM_DVE_ROW_BASE + len(dve_ops.OPS)
        dve_ops._SUB_OPCODE_FOR_NAME[name] = row
        shas = {}
        for ver in ("v3", "v4"):
            ds = DveOpSpec(name=name, opcode=row, uops=lower(spec, ver=ver),
                           rd1_en=True)
            shas[ver] = ds.sha(ver)
        op = dve_ops.DveOp(name, spec, subdim=False, uops_sha=shas)
        dve_ops.OPS.append(op)
        dve_ops.CUSTOM_DVE_SPECS[name] = spec
        return op

    csel = mk(
        "COMP_COEFF_SEL", select(Src0 > Src1, C0, C1),
        lambda in0, in1, s0, s1, imm2: np.where(in0 > in1, s0, s1).astype(np.float32),
    )
    d1sel = mk(
        "COMP_D1_SEL", select(Src0 > Src1, C0, C1) * Src0,
        lambda in0, in1, s0, s1, imm2: (np.where(in0 > in1, s0, s1) * in0).astype(np.float32),
    )
    return csel, d1sel


def _build_program(thr, ratio, att, rel, n_iter=N_ITER, nblk=NBLK, p=P, c=C, w=W):
    import concourse.bacc as bacc
    import concourse.mybir as mybir
    from concourse.ap import AP
    from concourse.tile import TileContext

    CSEL, D1SEL = _register_custom_ops()

    fp32 = mybir.dt.float32
    AF = mybir.ActivationFunctionType
    ALU = mybir.AluOpType

    shard = p * c
    fd = w + c
    assert fd % nblk == 0, (fd, nblk)
    bw = fd // nblk
    bounds = [i * bw for i in range(nblk + 1)]

    ln10 = math.log(10.0)
    k2 = 1.0 - 1.0 / ratio
    act_scale = -(20.0 * k2) / ln10   # gr_db = relu(act_scale*ln(|a|+eps) + act_bias)
    act_bias = thr * k2
    chalf = (att + rel) / 2.0
    cdiff = (att - rel) / 2.0
    exp_scale = -ln10 / 20.0

    nc = bacc.Bacc("TRN2", target_bir_lowering=False)

    def reg_const(val):
        val = float(val)
        key = (fp32, val)
        if key not in nc.const_aps.aps:
            t = nc.alloc_sbuf_tensor(f"const-f32-{val}", [128, 1], fp32)
            nc.gpsimd.memset(t.ap(), val)
            nc.const_aps.aps[key] = t.ap()

    reg_const(1e-5)
    reg_const(act_bias)
    nc.all_engine_barrier()

    ain = nc.dram_tensor("a_in", [shard + w], fp32, kind="ExternalInput")
    aout = nc.dram_tensor("a_out", [shard], fp32, kind="ExternalOutput")
    ain_h = ain.ap().tensor
    aout_h = aout.ap().tensor

    with TileContext(nc) as tc:
        with tc.tile_pool(name="pool", bufs=1) as pool:
            aud = pool.tile([p, fd], fp32, tag="aud")
            tA = pool.tile([p, fd], fp32, tag="tA")     # abs scratch, later sign / gain
            tB = pool.tile([p, fd], fp32, tag="tB")     # ln scratch, later s / out
            x = pool.tile([p, fd], fp32, tag="x")       # gain_reduction_db
            relt = pool.tile([p, fd], fp32, tag="relt")  # const release tile
            m = pool.tile([p, fd], fp32, tag="m")       # predictor scan
            coeff = pool.tile([p, fd], fp32, tag="coeff")
            d1 = pool.tile([p, fd], fp32, tag="d1")
            g = pool.tile([p, fd], fp32, tag="g")
            if n_iter > 1:
                g2 = pool.tile([p, fd], fp32, tag="g2")
            else:
                g2 = None
            scratch = pool.tile([p, 1], fp32, tag="scratch")

            # The hardware scan instruction (S2S2D2_STT encoding) has room
            # for very few semaphore waits; a 1-column vector copy "touches"
            # the scalar-engine output x right before each predictor scan so
            # the cross-engine wait lands on the copy instead.
            nc.gpsimd.memset(relt[:], rel)

            for b in range(nblk):
                c0, c1 = bounds[b], bounds[b + 1]
                blk = slice(c0, c1)
                # rows: aud[pp, col] = ain[pp*c + col]; rows overlap by w
                src = AP(ain_h, c0, [[c, p], [1, c1 - c0]])
                nc.sync.dma_start(out=aud[:, blk], in_=src)

                # front-end elementwise (scalar engine)
                nc.scalar.activation(tA[:, blk], aud[:, blk], AF.Abs)
                nc.scalar.activation(tB[:, blk], tA[:, blk], AF.Ln, bias=1e-5)
                nc.scalar.activation(x[:, blk], tB[:, blk], AF.Relu,
                                     bias=act_bias, scale=act_scale)

                # predictor scan: m[t] = max(rel*m[t-1], x[t])
                nc.vector.tensor_copy(scratch[:, 0:1], x[:, c1 - 1:c1])
                nc.vector.tensor_tensor_scan(
                    m[:, blk], relt[:, blk], x[:, blk],
                    initial=0.0 if b == 0 else m[:, c0 - 1:c0],
                    op0=ALU.mult, op1=ALU.max)

                prev = m
                cur = g
                for it in range(n_iter):
                    # coeff = select(x[t] > prev[t-1], att, rel)
                    # d1    = select(x[t] > prev[t-1], 1-att, 1-rel) * x[t]
                    if b == 0:
                        nc.vector.memset(coeff[:, 0:1], rel)
                        nc.vector.tensor_scalar_mul(d1[:, 0:1], x[:, 0:1], 1.0 - rel)
                        nc.vector._custom_dve(
                            CSEL, out=coeff[:, 1:c1], in0=x[:, 1:c1],
                            in1=prev[:, 0:c1 - 1], s0=att, s1=rel)
                        nc.vector._custom_dve(
                            D1SEL, out=d1[:, 1:c1], in0=x[:, 1:c1],
                            in1=prev[:, 0:c1 - 1], s0=1.0 - att, s1=1.0 - rel)
                    else:
                        nc.vector._custom_dve(
                            CSEL, out=coeff[:, blk], in0=x[:, blk],
                            in1=prev[:, c0 - 1:c1 - 1], s0=att, s1=rel)
                        nc.vector._custom_dve(
                            D1SEL, out=d1[:, blk], in0=x[:, blk],
                            in1=prev[:, c0 - 1:c1 - 1], s0=1.0 - att, s1=1.0 - rel)
                    nc.vector.tensor_tensor_scan(
                        cur[:, blk], coeff[:, blk], d1[:, blk],
                        initial=0.0 if b == 0 else cur[:, c0 - 1:c0],
                        op0=ALU.mult, op1=ALU.add)
                    prev, cur = cur, (g2 if prev is m else prev)

                gfin = prev
                # back-end: gain = exp(-ln10/20 * g); out = audio*gain
                v0 = max(c0, w)
                nc.scalar.activation(tA[:, v0:c1], gfin[:, v0:c1], AF.Exp,
                                     scale=exp_scale)
                nc.vector.tensor_tensor(
                    tB[:, v0:c1], aud[:, v0:c1], tA[:, v0:c1], op=ALU.mult)
                dst = AP(aout_h, v0 - w, [[c, p], [1, c1 - v0]])
                nc.sync.dma_start(out=dst, in_=tB[:, v0:c1])

    if not nc.is_finalized():
        nc.finalize()
    return nc


_CACHE = {}


def _get_program(thr, ratio, att, rel):
    key = (float(thr), float(ratio), float(att), float(rel), N_ITER, NBLK)
    if key not in _CACHE:
        _CACHE[key] = _build_program(*key[:4], n_iter=N_ITER, nblk=NBLK)
    return _CACHE[key]


def kernel(audio, threshold, ratio, attack, release):
    from concourse.bass_utils import run_bass_kernel_spmd

    audio = np.asarray(audio, dtype=np.float32)
    assert audio.shape == (T_TOTAL,), audio.shape
    thr = float(np.asarray(threshold))
    rat = float(np.asarray(ratio))
    att = float(np.asarray(attack))
    rel = float(np.asarray(release))

    nc = _get_program(thr, rat, att, rel)

    padded = np.concatenate([np.full(W, PAD_VAL, dtype=np.float32), audio])
    in_maps = [
        {"a_in": padded[cid * SHARD: cid * SHARD + SHARD + W]}
        for cid in range(NCORES)
    ]
    res = run_bass_kernel_spmd(nc, in_maps, list(range(NCORES)))
    out = np.concatenate([res.results[cid]["a_out"] for cid in range(NCORES)])
    return out.astype(np.float32)

